# revision 14
# baseline (speedup 1.0000x reference)
"""Trainium2 Bass kernel for nn_MultiHeadAttention (B=2, S=2048, H=1024, 16 heads).

Sharding (Megatron-style tensor parallel over heads):
  - core c owns heads {2c, 2c+1} (hidden dims 128c..128c+127) for BOTH batches.
  - Wq/Wk/Wv row-sharded, Wo column-sharded. Each core emits TWO full-shape
    per-head UNNORMALIZED partial outputs (bf16) plus the softmax denominator
    rows; the host divides by the denominators, sums the 16 partials, and adds
    bo + bv @ Wo.T.

v4 design (vs 380us v1 baseline):
  - all matmul operands bf16 (psum stays f32); x shipped bf16 (halves DMA).
  - scores row-tiled: both heads' K=64 matmuls run concurrently on PE
    row-groups (tile_position (0,0)/(64,0)) -> 2x on score walls.
  - PV col-tiled: head0 -> psum partitions 0-63, head1 -> 64-127 of ONE
    [128,512] bank; denominators via M=1 ones-matmuls at col pos (0,0)/(0,32).
  - V produced token-major directly (x-stationary matmuls), no transposes.
  - softmax normalization moved to the HOST: no reciprocal / row-broadcast on
    device; O-projection runs per head as row-tiled concurrent K=64 pairs on
    the unnormalized attention numerator (same PE cost as the fused version).
  - O-projection of q-tile N is emitted after the score/exp/PV chunks of
    q-tile N+1, so its matmuls/casts/DMAs fill the ACT-bound softmax window.
  - PSUM: 3x[128,1024] score ring (also phase-1 and O-proj psums) + pv + dn.
"""

import numpy as np

HIDDEN = 1024
HEADS = 16
HD = 64
B, S = 2, 2048
NTOK = B * S            # 4096
NCORES = 8
HSL = HIDDEN // NCORES  # 128 hidden dims per core (2 heads)
P = 128
FCH = HIDDEN // P       # 8 contraction chunks
TOK_TILE = 512
NTT = NTOK // TOK_TILE  # 8 token tiles (4 per batch)
QT_W = 512
NQT = S // QT_W         # 4 q tiles per batch
NKC = S // P            # 16 kv chunks per batch

_CACHE = {}


def _build_bass():
    import concourse.bacc as bacc
    import concourse.mybir as mybir
    import concourse.tile as tile

    f32 = mybir.dt.float32
    bf16 = mybir.dt.bfloat16
    Exp = mybir.ActivationFunctionType.Exp
    Ident = mybir.ActivationFunctionType.Identity

    nc = bacc.Bacc("TRN2", target_bir_lowering=False, debug=False,
                   num_devices=NCORES)

    xT = nc.dram_tensor("xT", [HIDDEN, NTOK], bf16, kind="ExternalInput").ap()
    wqT = nc.dram_tensor("wqT", [HIDDEN, HSL], bf16, kind="ExternalInput").ap()
    wkT = nc.dram_tensor("wkT", [HIDDEN, HSL], bf16, kind="ExternalInput").ap()
    wvT = nc.dram_tensor("wvT", [HIDDEN, HSL], bf16, kind="ExternalInput").ap()
    woT = nc.dram_tensor("woT", [HSL, HIDDEN], bf16, kind="ExternalInput").ap()
    bq = nc.dram_tensor("bq", [HSL, 1], f32, kind="ExternalInput").ap()
    bk = nc.dram_tensor("bk", [HSL, 1], f32, kind="ExternalInput").ap()
    onesd = nc.dram_tensor("onesd", [P, 1], bf16, kind="ExternalInput").ap()
    outT0 = nc.dram_tensor("outT0", [HIDDEN, NTOK], bf16,
                           kind="ExternalOutput").ap()
    outT1 = nc.dram_tensor("outT1", [HIDDEN, NTOK], bf16,
                           kind="ExternalOutput").ap()
    dnout = nc.dram_tensor("dnout", [2, NTOK], f32,
                           kind="ExternalOutput").ap()

    with tile.TileContext(nc) as tc:
        import contextlib
        ctx = contextlib.ExitStack()
        with ctx:
            wpool = ctx.enter_context(tc.tile_pool(name="w", bufs=1))
            xpool = ctx.enter_context(tc.tile_pool(name="x", bufs=2))
            qkpool = ctx.enter_context(tc.tile_pool(name="qk", bufs=1))
            vpool = ctx.enter_context(tc.tile_pool(name="v", bufs=1))
            ppool = ctx.enter_context(tc.tile_pool(name="p", bufs=3))
            spool = ctx.enter_context(tc.tile_pool(name="scl", bufs=2))
            apool = ctx.enter_context(tc.tile_pool(name="attn", bufs=2))
            opool = ctx.enter_context(tc.tile_pool(name="osb", bufs=4))
            sps = ctx.enter_context(
                tc.tile_pool(name="sps", bufs=2, space="PSUM"))
            aux = ctx.enter_context(
                tc.tile_pool(name="aux", bufs=1, space="PSUM"))

            # ---- weights / biases ----
            wq_sb = wpool.tile([P, FCH, HSL], bf16)
            wk_sb = wpool.tile([P, FCH, HSL], bf16)
            wv_sb = wpool.tile([P, FCH, HSL], bf16)
            wo_sb = wpool.tile([P, HIDDEN], bf16)
            bq_sb = wpool.tile([P, 1], f32)
            bk_sb = wpool.tile([P, 1], f32)
            ones_sb = wpool.tile([P, 1], bf16)
            nc.sync.dma_start(wq_sb, wqT.rearrange("(c p) m -> p c m", p=P))
            nc.sync.dma_start(wk_sb, wkT.rearrange("(c p) m -> p c m", p=P))
            nc.sync.dma_start(wv_sb, wvT.rearrange("(c p) m -> p c m", p=P))
            nc.sync.dma_start(wo_sb, woT)
            nc.sync.dma_start(bq_sb, bq)
            nc.sync.dma_start(bk_sb, bk)
            nc.sync.dma_start(ones_sb, onesd)

            qt_sb = qkpool.tile([P, NTOK], bf16)
            kt_sb = qkpool.tile([P, NTOK], bf16)
            v_sb = vpool.tile([P, B * NKC, HSL], bf16)

            _sec_ctr = [0]

            def phase1_sections(tt):
                """Return a list of closures, each a ~1-2us slice of the
                projection work for token tile tt; psum comes from the aux
                o0/o1 slots so the score ring stays free."""
                tsl = slice(tt * TOK_TILE, (tt + 1) * TOK_TILE)
                state = {}

                def aux_ps(nm):
                    tg = "o0" if _sec_ctr[0] % 2 == 0 else "o1"
                    _sec_ctr[0] += 1
                    return aux.tile([P, QT_W], f32, tag=tg, name=nm)

                def sec_q():
                    x_t = xpool.tile([P, FCH, TOK_TILE], bf16, tag="x",
                                     name=f"x{tt}")
                    nc.sync.dma_start(
                        x_t, xT[:, tsl].rearrange("(c p) n -> p c n", p=P))
                    state["x"] = x_t
                    q_ps = aux_ps(f"qp{tt}")
                    for f in range(FCH):
                        nc.tensor.matmul(q_ps, wq_sb[:, f, :], x_t[:, f, :],
                                         start=(f == 0), stop=(f == FCH - 1))
                    nc.vector.tensor_scalar_add(qt_sb[:, tsl], q_ps, bq_sb)

                def sec_k():
                    x_t = state["x"]
                    k_ps = aux_ps(f"kp{tt}")
                    for f in range(FCH):
                        nc.tensor.matmul(k_ps, wk_sb[:, f, :], x_t[:, f, :],
                                         start=(f == 0), stop=(f == FCH - 1))
                    nc.vector.tensor_scalar_add(kt_sb[:, tsl], k_ps, bk_sb)

                def sec_v(sub):
                    x_t = state["x"]
                    v_ps = aux_ps(f"vp{tt}{sub}")
                    ssl = slice(sub * P, (sub + 1) * P)
                    for f in range(FCH):
                        nc.tensor.matmul(v_ps[:, 0:HSL],
                                         x_t[:, f, ssl], wv_sb[:, f, :],
                                         start=(f == 0), stop=(f == FCH - 1))
                    gc = (tt * TOK_TILE) // P + sub
                    nc.vector.tensor_copy(v_sb[:, gc, :], v_ps[:, 0:HSL])

                return [sec_q, sec_k] +                     [(lambda s=s: sec_v(s)) for s in range(TOK_TILE // P)]

            def phase1_tile(tt):
                tsl = slice(tt * TOK_TILE, (tt + 1) * TOK_TILE)
                x_t = xpool.tile([P, FCH, TOK_TILE], bf16, tag="x",
                                 name=f"x{tt}")
                nc.sync.dma_start(
                    x_t, xT[:, tsl].rearrange("(c p) n -> p c n", p=P))

                q_ps = sps.tile([P, TOK_TILE], f32, tag="s", name=f"qp{tt}")
                for f in range(FCH):
                    nc.tensor.matmul(q_ps, wq_sb[:, f, :], x_t[:, f, :],
                                     start=(f == 0), stop=(f == FCH - 1))
                nc.scalar.activation(qt_sb[:, tsl], q_ps, Ident, bias=bq_sb)

                k_ps = sps.tile([P, TOK_TILE], f32, tag="s", name=f"kp{tt}")
                for f in range(FCH):
                    nc.tensor.matmul(k_ps, wk_sb[:, f, :], x_t[:, f, :],
                                     start=(f == 0), stop=(f == FCH - 1))
                nc.scalar.activation(kt_sb[:, tsl], k_ps, Ident, bias=bk_sb)

                # V token-major: out[tok128, 128dims] = x_chunk.T @ wv_chunk
                for sub in range(TOK_TILE // P):
                    v_ps = sps.tile([P, TOK_TILE], f32, tag="s",
                                    name=f"vp{tt}{sub}")
                    ssl = slice(sub * P, (sub + 1) * P)
                    for f in range(FCH):
                        nc.tensor.matmul(v_ps[:, 0:HSL],
                                         x_t[:, f, ssl], wv_sb[:, f, :],
                                         start=(f == 0), stop=(f == FCH - 1))
                    gc = (tt * TOK_TILE) // P + sub
                    nc.vector.tensor_copy(v_sb[:, gc, :], v_ps[:, 0:HSL])

            def oproj_pair(b, qt, u_sb, f, final=False):
                qsl = slice(b * S + qt * QT_W, b * S + (qt + 1) * QT_W)
                fsl = slice(f * P, (f + 1) * P)
                o0 = aux.tile([P, QT_W], f32, tag="o0", name=f"o0{b}{qt}{f}")
                o1 = aux.tile([P, QT_W], f32, tag="o1", name=f"o1{b}{qt}{f}")
                nc.tensor.matmul(o0, wo_sb[0:HD, fsl], u_sb[0:HD, :],
                                 start=True, stop=True, tile_position=(0, 0))
                nc.tensor.matmul(o1, wo_sb[HD:P, fsl], u_sb[HD:P, :],
                                 start=True, stop=True, tile_position=(HD, 0))
                ob0 = opool.tile([P, QT_W], bf16, tag="ob",
                                 name=f"ob0{b}{qt}{f}")
                ob1 = opool.tile([P, QT_W], bf16, tag="ob",
                                 name=f"ob1{b}{qt}{f}")
                if final:
                    nc.scalar.activation(ob0, o0,
                                         mybir.ActivationFunctionType.Copy)
                else:
                    nc.vector.tensor_copy(ob0, o0)
                nc.vector.tensor_copy(ob1, o1)
                nc.sync.dma_start(outT0[fsl, qsl], ob0)
                nc.sync.dma_start(outT1[fsl, qsl], ob1)

            def phase2_chunks(b, qt, prev, deadlines):
                """scores -> exp -> PV/denominator chunks with the previous
                q-tile's O-projection interleaved; evacuate the unnormalized
                numerator + denominator rows. Returns u_sb."""
                qsl = slice(b * S + qt * QT_W, b * S + (qt + 1) * QT_W)
                pv01 = aux.tile([P, QT_W], f32, tag="pv", name=f"pv{b}{qt}")
                dn = aux.tile([P, QT_W], f32, tag="dn", name=f"dn{b}{qt}")
                p_tiles = {}

                def emit_scores(c):
                    gc = b * NKC + c
                    ksl = slice(gc * P, (gc + 1) * P)
                    s_c = sps.tile([P, 2 * QT_W], f32, tag="s",
                                   name=f"s{b}{qt}{c}")
                    nc.tensor.matmul(s_c[:, 0:QT_W],
                                     kt_sb[0:HD, ksl], qt_sb[0:HD, qsl],
                                     start=True, stop=True,
                                     tile_position=(0, 0))
                    nc.tensor.matmul(s_c[:, QT_W:2 * QT_W],
                                     kt_sb[HD:P, ksl], qt_sb[HD:P, qsl],
                                     start=True, stop=True,
                                     tile_position=(HD, 0))
                    p_c = ppool.tile([P, 2 * QT_W], bf16, tag="p",
                                     name=f"p{b}{qt}{c}")
                    nc.scalar.activation(p_c, s_c, Exp, scale=0.125)
                    p_tiles[c] = p_c

                def emit_pv(c):
                    gc = b * NKC + c
                    p_c = p_tiles.pop(c)
                    st = dict(start=(c == 0), stop=(c == NKC - 1),
                              skip_group_check=True)
                    nc.tensor.matmul(pv01[0:HD, :], v_sb[:, gc, 0:HD],
                                     p_c[:, 0:QT_W],
                                     tile_position=(0, 0), **st)
                    nc.tensor.matmul(pv01[HD:P, :], v_sb[:, gc, HD:P],
                                     p_c[:, QT_W:2 * QT_W],
                                     tile_position=(0, HD), **st)
                    nc.tensor.matmul(dn[0:1, :], ones_sb, p_c[:, 0:QT_W],
                                     tile_position=(0, 0), **st)
                    nc.tensor.matmul(dn[32:33, :], ones_sb,
                                     p_c[:, QT_W:2 * QT_W],
                                     tile_position=(0, 32), **st)

                # software-pipelined: scores/exp run one chunk ahead of
                # PV/denominator so exp(c+1) never waits on chunk c's tail
                emit_scores(0)
                for c in range(NKC):
                    if c + 1 < NKC:
                        while deadlines and deadlines[0][0] <= b * NKC + c + 1:
                            deadlines.pop(0)[1]()
                        emit_scores(c + 1)
                    if prev is not None and c % 2 == 1:
                        oproj_pair(prev[0], prev[1], prev[2], c // 2)
                    if deadlines and c % 2 == 0:
                        deadlines.pop(0)[1]()
                    emit_pv(c)

                rows = spool.tile([33, QT_W], f32, tag="rd",
                                  name=f"rd{b}{qt}")
                nc.vector.tensor_copy(rows[0:1, :], dn[0:1, :])
                nc.vector.tensor_copy(rows[32:33, :], dn[32:33, :])
                u_sb = apool.tile([P, QT_W], bf16, tag="u", name=f"u{b}{qt}")
                nc.scalar.activation(u_sb, pv01,
                                     mybir.ActivationFunctionType.Copy)
                nc.sync.dma_start(dnout[0:1, qsl], rows[0:1, :])
                nc.sync.dma_start(dnout[1:2, qsl], rows[32:33, :])
                return u_sb

            phase1_tile(0)
            deadlines = []
            for tt in range(1, NTT):
                for sec in phase1_sections(tt):
                    deadlines.append((4 * tt, sec))
            prev = None
            for b in range(B):
                for qt in range(NQT):
                    u = phase2_chunks(b, qt, prev, deadlines)
                    prev = (b, qt, u)
            assert not deadlines
            for f in range(FCH):
                oproj_pair(prev[0], prev[1], prev[2], f, final=True)

    nc.compile()
    return nc


def _shard_inputs(x, Wq, bq, Wk, bk, Wv, bv, Wo, bo):
    import ml_dtypes
    bf = ml_dtypes.bfloat16
    xT = np.ascontiguousarray(
        np.asarray(x).reshape(NTOK, HIDDEN).T).astype(bf)
    ones = np.ones((P, 1), dtype=bf)
    in_maps = []
    for c in range(NCORES):
        rs = slice(HSL * c, HSL * (c + 1))
        in_maps.append({
            "xT": xT,
            "wqT": np.ascontiguousarray(Wq[rs].T).astype(bf),
            "wkT": np.ascontiguousarray(Wk[rs].T).astype(bf),
            "wvT": np.ascontiguousarray(Wv[rs].T).astype(bf),
            "woT": np.ascontiguousarray(Wo[:, rs].T).astype(bf),
            "bq": np.ascontiguousarray(
                bq[rs].reshape(HSL, 1).astype(np.float32)),
            "bk": np.ascontiguousarray(
                bk[rs].reshape(HSL, 1).astype(np.float32)),
            "onesd": ones,
        })
    return in_maps


def kernel(x, Wq, bq, Wk, bk, Wv, bv, Wo, bo):
    from concourse.bass_utils import run_bass_kernel_spmd

    if "nc" not in _CACHE:
        _CACHE["nc"] = _build_bass()
    nc = _CACHE["nc"]

    in_maps = _shard_inputs(x, Wq, bq, Wk, bk, Wv, bv, Wo, bo)
    res = run_bass_kernel_spmd(nc, in_maps, core_ids=list(range(NCORES)))
    kernel._last_results = res

    acc = np.zeros((HIDDEN, NTOK), dtype=np.float32)
    for r in res.results:
        dn = np.asarray(r["dnout"]).astype(np.float32)
        acc += np.asarray(r["outT0"]).astype(np.float32) / dn[0:1, :]
        acc += np.asarray(r["outT1"]).astype(np.float32) / dn[1:2, :]
    out = acc.T.reshape(B, S, HIDDEN)
    out += (bo + bv @ Wo.T).astype(np.float32)
    return out.astype(np.float32)


# revision 15
# speedup vs baseline: 1.2224x; 1.2224x over previous
"""Trainium2 Bass kernel for nn_MultiHeadAttention (B=2, S=2048, H=1024, 16 heads).

Sharding (Megatron-style tensor parallel over heads):
  - core c owns heads {2c, 2c+1} (hidden dims 128c..128c+127) for BOTH batches.
  - Wq/Wk/Wv row-sharded, Wo column-sharded. Each core emits TWO full-shape
    per-head UNNORMALIZED partial outputs (bf16) plus the softmax denominator
    rows; the host divides by the denominators, sums the 16 partials, and adds
    bo + bv @ Wo.T.

v4 design (vs 380us v1 baseline):
  - all matmul operands bf16 (psum stays f32); x shipped bf16 (halves DMA).
  - scores row-tiled: both heads' K=64 matmuls run concurrently on PE
    row-groups (tile_position (0,0)/(64,0)) -> 2x on score walls.
  - PV col-tiled: head0 -> psum partitions 0-63, head1 -> 64-127 of ONE
    [128,512] bank; denominators via M=1 ones-matmuls at col pos (0,0)/(0,32).
  - V produced token-major directly (x-stationary matmuls), no transposes.
  - softmax normalization moved to the HOST: no reciprocal / row-broadcast on
    device; O-projection runs per head as row-tiled concurrent K=64 pairs on
    the unnormalized attention numerator (same PE cost as the fused version).
  - O-projection of q-tile N is emitted after the score/exp/PV chunks of
    q-tile N+1, so its matmuls/casts/DMAs fill the ACT-bound softmax window.
  - PSUM: 3x[128,1024] score ring (also phase-1 and O-proj psums) + pv + dn.
"""

import numpy as np

HIDDEN = 1024
HEADS = 16
HD = 64
B, S = 2, 2048
NTOK = B * S            # 4096
NCORES = 8
HSL = HIDDEN // NCORES  # 128 hidden dims per core (2 heads)
P = 128
FCH = HIDDEN // P       # 8 contraction chunks
TOK_TILE = 512
NTT = NTOK // TOK_TILE  # 8 token tiles (4 per batch)
QT_W = 512
NQT = S // QT_W         # 4 q tiles per batch
NKC = S // P            # 16 kv chunks per batch

_CACHE = {}


def _build_bass():
    import concourse.bacc as bacc
    import concourse.mybir as mybir
    import concourse.tile as tile

    f32 = mybir.dt.float32
    bf16 = mybir.dt.bfloat16
    Exp = mybir.ActivationFunctionType.Exp
    Ident = mybir.ActivationFunctionType.Identity

    nc = bacc.Bacc("TRN2", target_bir_lowering=False, debug=False,
                   num_devices=NCORES)

    xT = nc.dram_tensor("xT", [HIDDEN, NTOK], bf16, kind="ExternalInput").ap()
    wqT = nc.dram_tensor("wqT", [HIDDEN, HSL], bf16, kind="ExternalInput").ap()
    wkT = nc.dram_tensor("wkT", [HIDDEN, HSL], bf16, kind="ExternalInput").ap()
    wvT = nc.dram_tensor("wvT", [HIDDEN, HSL], bf16, kind="ExternalInput").ap()
    woT = nc.dram_tensor("woT", [HSL, HIDDEN], bf16, kind="ExternalInput").ap()
    bq = nc.dram_tensor("bq", [HSL, 1], f32, kind="ExternalInput").ap()
    bk = nc.dram_tensor("bk", [HSL, 1], f32, kind="ExternalInput").ap()
    onesd = nc.dram_tensor("onesd", [P, 1], bf16, kind="ExternalInput").ap()
    outT0 = nc.dram_tensor("outT0", [HIDDEN, NTOK], bf16,
                           kind="ExternalOutput").ap()
    outT1 = nc.dram_tensor("outT1", [HIDDEN, NTOK], bf16,
                           kind="ExternalOutput").ap()
    dnout = nc.dram_tensor("dnout", [2, NTOK], f32,
                           kind="ExternalOutput").ap()

    with tile.TileContext(nc) as tc:
        import contextlib
        ctx = contextlib.ExitStack()
        with ctx:
            wpool = ctx.enter_context(tc.tile_pool(name="w", bufs=1))
            xpool = ctx.enter_context(tc.tile_pool(name="x", bufs=3))
            qkpool = ctx.enter_context(tc.tile_pool(name="qk", bufs=1))
            vpool = ctx.enter_context(tc.tile_pool(name="v", bufs=1))
            ppool = ctx.enter_context(tc.tile_pool(name="p", bufs=4))
            spool = ctx.enter_context(tc.tile_pool(name="scl", bufs=2))
            apool = ctx.enter_context(tc.tile_pool(name="attn", bufs=2))
            opool = ctx.enter_context(tc.tile_pool(name="osb", bufs=6))
            sps = ctx.enter_context(
                tc.tile_pool(name="sps", bufs=2, space="PSUM"))
            aux = ctx.enter_context(
                tc.tile_pool(name="aux", bufs=1, space="PSUM"))

            # prefetch tile 0's activations before the weight DMAs so
            # the first projection matmuls start as early as possible
            x_t0 = xpool.tile([P, FCH, TOK_TILE], bf16, tag="x", name="x0")
            nc.sync.dma_start(
                x_t0, xT[:, 0:TOK_TILE].rearrange("(c p) n -> p c n", p=P))

            # ---- weights / biases ----
            wq_sb = wpool.tile([P, FCH, HSL], bf16)
            wk_sb = wpool.tile([P, FCH, HSL], bf16)
            wv_sb = wpool.tile([P, FCH, HSL], bf16)
            wo_sb = wpool.tile([P, HIDDEN], bf16)
            bq_sb = wpool.tile([P, 1], f32)
            bk_sb = wpool.tile([P, 1], f32)
            ones_sb = wpool.tile([P, 1], bf16)
            nc.sync.dma_start(wq_sb, wqT.rearrange("(c p) m -> p c m", p=P))
            nc.sync.dma_start(wk_sb, wkT.rearrange("(c p) m -> p c m", p=P))
            nc.sync.dma_start(wv_sb, wvT.rearrange("(c p) m -> p c m", p=P))
            nc.sync.dma_start(wo_sb, woT)
            nc.sync.dma_start(bq_sb, bq)
            nc.sync.dma_start(bk_sb, bk)
            nc.sync.dma_start(ones_sb, onesd)

            qt_sb = qkpool.tile([P, NTOK], bf16)
            kt_sb = qkpool.tile([P, NTOK], bf16)
            v_sb = vpool.tile([P, B * NKC, HSL], bf16)

            _sec_ctr = [0]

            def phase1_sections(tt):
                """Return a list of closures, each a ~1-2us slice of the
                projection work for token tile tt; psum comes from the aux
                o0/o1 slots so the score ring stays free."""
                tsl = slice(tt * TOK_TILE, (tt + 1) * TOK_TILE)
                state = {}

                def aux_ps(nm):
                    tg = "o0" if _sec_ctr[0] % 2 == 0 else "o1"
                    _sec_ctr[0] += 1
                    return aux.tile([P, QT_W], f32, tag=tg, name=nm)

                def sec_q():
                    x_t = xpool.tile([P, FCH, TOK_TILE], bf16, tag="x",
                                     name=f"x{tt}")
                    nc.sync.dma_start(
                        x_t, xT[:, tsl].rearrange("(c p) n -> p c n", p=P))
                    state["x"] = x_t
                    q_ps = aux_ps(f"qp{tt}")
                    for f in range(FCH):
                        nc.tensor.matmul(q_ps, wq_sb[:, f, :], x_t[:, f, :],
                                         start=(f == 0), stop=(f == FCH - 1))
                    nc.vector.tensor_scalar_add(qt_sb[:, tsl], q_ps, bq_sb)

                def sec_k():
                    x_t = state["x"]
                    k_ps = aux_ps(f"kp{tt}")
                    for f in range(FCH):
                        nc.tensor.matmul(k_ps, wk_sb[:, f, :], x_t[:, f, :],
                                         start=(f == 0), stop=(f == FCH - 1))
                    nc.vector.tensor_scalar_add(kt_sb[:, tsl], k_ps, bk_sb)

                def sec_v(sub):
                    x_t = state["x"]
                    v_ps = aux_ps(f"vp{tt}{sub}")
                    ssl = slice(sub * P, (sub + 1) * P)
                    for f in range(FCH):
                        nc.tensor.matmul(v_ps[:, 0:HSL],
                                         x_t[:, f, ssl], wv_sb[:, f, :],
                                         start=(f == 0), stop=(f == FCH - 1))
                    gc = (tt * TOK_TILE) // P + sub
                    nc.vector.tensor_copy(v_sb[:, gc, :], v_ps[:, 0:HSL])

                return [sec_q, sec_k] +                     [(lambda s=s: sec_v(s)) for s in range(TOK_TILE // P)]

            def phase1_tile(tt):
                tsl = slice(tt * TOK_TILE, (tt + 1) * TOK_TILE)
                if tt == 0:
                    x_t = x_t0
                else:
                    x_t = xpool.tile([P, FCH, TOK_TILE], bf16, tag="x",
                                     name=f"x{tt}")
                    nc.sync.dma_start(
                        x_t, xT[:, tsl].rearrange("(c p) n -> p c n", p=P))

                q_ps = sps.tile([P, TOK_TILE], f32, tag="s", name=f"qp{tt}")
                for f in range(FCH):
                    nc.tensor.matmul(q_ps, wq_sb[:, f, :], x_t[:, f, :],
                                     start=(f == 0), stop=(f == FCH - 1))
                nc.scalar.activation(qt_sb[:, tsl], q_ps, Ident, bias=bq_sb)

                k_ps = sps.tile([P, TOK_TILE], f32, tag="s", name=f"kp{tt}")
                for f in range(FCH):
                    nc.tensor.matmul(k_ps, wk_sb[:, f, :], x_t[:, f, :],
                                     start=(f == 0), stop=(f == FCH - 1))
                nc.scalar.activation(kt_sb[:, tsl], k_ps, Ident, bias=bk_sb)

                # V token-major: out[tok128, 128dims] = x_chunk.T @ wv_chunk
                for sub in range(TOK_TILE // P):
                    v_ps = sps.tile([P, TOK_TILE], f32, tag="s",
                                    name=f"vp{tt}{sub}")
                    ssl = slice(sub * P, (sub + 1) * P)
                    for f in range(FCH):
                        nc.tensor.matmul(v_ps[:, 0:HSL],
                                         x_t[:, f, ssl], wv_sb[:, f, :],
                                         start=(f == 0), stop=(f == FCH - 1))
                    gc = (tt * TOK_TILE) // P + sub
                    nc.vector.tensor_copy(v_sb[:, gc, :], v_ps[:, 0:HSL])

            def oproj_pair(b, qt, u_sb, f, final=False):
                qsl = slice(b * S + qt * QT_W, b * S + (qt + 1) * QT_W)
                fsl = slice(f * P, (f + 1) * P)
                o0 = aux.tile([P, QT_W], f32, tag="o0", name=f"o0{b}{qt}{f}")
                o1 = aux.tile([P, QT_W], f32, tag="o1", name=f"o1{b}{qt}{f}")
                nc.tensor.matmul(o0, wo_sb[0:HD, fsl], u_sb[0:HD, :],
                                 start=True, stop=True, tile_position=(0, 0))
                nc.tensor.matmul(o1, wo_sb[HD:P, fsl], u_sb[HD:P, :],
                                 start=True, stop=True, tile_position=(HD, 0))
                ob0 = opool.tile([P, QT_W], bf16, tag="ob",
                                 name=f"ob0{b}{qt}{f}")
                ob1 = opool.tile([P, QT_W], bf16, tag="ob",
                                 name=f"ob1{b}{qt}{f}")
                if final:
                    nc.scalar.activation(ob0, o0,
                                         mybir.ActivationFunctionType.Copy)
                else:
                    nc.vector.tensor_copy(ob0, o0)
                nc.vector.tensor_copy(ob1, o1)
                nc.sync.dma_start(outT0[fsl, qsl], ob0)
                nc.sync.dma_start(outT1[fsl, qsl], ob1)

            def phase2_chunks(b, qt, prev, deadlines):
                """scores -> exp -> PV/denominator chunks with the previous
                q-tile's O-projection interleaved; evacuate the unnormalized
                numerator + denominator rows. Returns u_sb."""
                qsl = slice(b * S + qt * QT_W, b * S + (qt + 1) * QT_W)
                pv01 = aux.tile([P, QT_W], f32, tag="pv", name=f"pv{b}{qt}")
                dn = aux.tile([P, QT_W], f32, tag="dn", name=f"dn{b}{qt}")
                p_tiles = {}

                def emit_scores(c):
                    gc = b * NKC + c
                    ksl = slice(gc * P, (gc + 1) * P)
                    s_c = sps.tile([P, 2 * QT_W], f32, tag="s",
                                   name=f"s{b}{qt}{c}")
                    nc.tensor.matmul(s_c[:, 0:QT_W],
                                     kt_sb[0:HD, ksl], qt_sb[0:HD, qsl],
                                     start=True, stop=True,
                                     tile_position=(0, 0))
                    nc.tensor.matmul(s_c[:, QT_W:2 * QT_W],
                                     kt_sb[HD:P, ksl], qt_sb[HD:P, qsl],
                                     start=True, stop=True,
                                     tile_position=(HD, 0))
                    p_c = ppool.tile([P, 2 * QT_W], bf16, tag="p",
                                     name=f"p{b}{qt}{c}")
                    nc.scalar.activation(p_c, s_c, Exp, scale=0.125)
                    p_tiles[c] = p_c

                def emit_pv(c):
                    gc = b * NKC + c
                    p_c = p_tiles.pop(c)
                    st = dict(start=(c == 0), stop=(c == NKC - 1),
                              skip_group_check=True)
                    nc.tensor.matmul(pv01[0:HD, :], v_sb[:, gc, 0:HD],
                                     p_c[:, 0:QT_W],
                                     tile_position=(0, 0), **st)
                    nc.tensor.matmul(pv01[HD:P, :], v_sb[:, gc, HD:P],
                                     p_c[:, QT_W:2 * QT_W],
                                     tile_position=(0, HD), **st)
                    nc.tensor.matmul(dn[0:1, :], ones_sb, p_c[:, 0:QT_W],
                                     tile_position=(0, 0), **st)
                    nc.tensor.matmul(dn[32:33, :], ones_sb,
                                     p_c[:, QT_W:2 * QT_W],
                                     tile_position=(0, 32), **st)

                # software-pipelined: scores/exp run one chunk ahead of
                # PV/denominator so exp(c+1) never waits on chunk c's tail
                emit_scores(0)
                for c in range(NKC):
                    if c + 1 < NKC:
                        while deadlines and deadlines[0][0] <= b * NKC + c + 1:
                            deadlines.pop(0)[1]()
                        emit_scores(c + 1)
                    if prev is not None and c % 2 == 1:
                        oproj_pair(prev[0], prev[1], prev[2], c // 2)
                    if deadlines and c % 2 == 0:
                        deadlines.pop(0)[1]()
                    emit_pv(c)

                rows = spool.tile([33, QT_W], f32, tag="rd",
                                  name=f"rd{b}{qt}")
                nc.vector.tensor_copy(rows[0:1, :], dn[0:1, :])
                nc.vector.tensor_copy(rows[32:33, :], dn[32:33, :])
                u_sb = apool.tile([P, QT_W], bf16, tag="u", name=f"u{b}{qt}")
                nc.scalar.activation(u_sb, pv01,
                                     mybir.ActivationFunctionType.Copy)
                nc.sync.dma_start(dnout[0:1, qsl], rows[0:1, :])
                nc.sync.dma_start(dnout[1:2, qsl], rows[32:33, :])
                return u_sb

            phase1_tile(0)
            deadlines = []
            for tt in range(1, NTT):
                for sec in phase1_sections(tt):
                    deadlines.append((4 * tt, sec))
            prev = None
            for b in range(B):
                for qt in range(NQT):
                    u = phase2_chunks(b, qt, prev, deadlines)
                    prev = (b, qt, u)
            assert not deadlines
            for f in range(FCH):
                oproj_pair(prev[0], prev[1], prev[2], f, final=True)

    nc.compile()
    return nc


def _shard_inputs(x, Wq, bq, Wk, bk, Wv, bv, Wo, bo):
    import ml_dtypes
    bf = ml_dtypes.bfloat16
    xT = np.ascontiguousarray(
        np.asarray(x).reshape(NTOK, HIDDEN).T).astype(bf)
    ones = np.ones((P, 1), dtype=bf)
    in_maps = []
    for c in range(NCORES):
        rs = slice(HSL * c, HSL * (c + 1))
        in_maps.append({
            "xT": xT,
            "wqT": np.ascontiguousarray(Wq[rs].T).astype(bf),
            "wkT": np.ascontiguousarray(Wk[rs].T).astype(bf),
            "wvT": np.ascontiguousarray(Wv[rs].T).astype(bf),
            "woT": np.ascontiguousarray(Wo[:, rs].T).astype(bf),
            "bq": np.ascontiguousarray(
                bq[rs].reshape(HSL, 1).astype(np.float32)),
            "bk": np.ascontiguousarray(
                bk[rs].reshape(HSL, 1).astype(np.float32)),
            "onesd": ones,
        })
    return in_maps


def kernel(x, Wq, bq, Wk, bk, Wv, bv, Wo, bo):
    from concourse.bass_utils import run_bass_kernel_spmd

    if "nc" not in _CACHE:
        _CACHE["nc"] = _build_bass()
    nc = _CACHE["nc"]

    in_maps = _shard_inputs(x, Wq, bq, Wk, bk, Wv, bv, Wo, bo)
    res = run_bass_kernel_spmd(nc, in_maps, core_ids=list(range(NCORES)))
    kernel._last_results = res

    acc = np.zeros((HIDDEN, NTOK), dtype=np.float32)
    for r in res.results:
        dn = np.asarray(r["dnout"]).astype(np.float32)
        acc += np.asarray(r["outT0"]).astype(np.float32) / dn[0:1, :]
        acc += np.asarray(r["outT1"]).astype(np.float32) / dn[1:2, :]
    out = acc.T.reshape(B, S, HIDDEN)
    out += (bo + bv @ Wo.T).astype(np.float32)
    return out.astype(np.float32)
